# revision 2
# baseline (speedup 1.0000x reference)
"""GAT (3-layer, PPI-style) forward on 8 Trainium2 NeuronCores — v2.

Strategy (graph/data parallel):
- Host: append self-loops, build per-(core, group-of-128-dsts) hybrid CSR-T
  edge layout: identity chunks (slot (k,p) = k-th in-edge of dst-local p,
  k < K1) + overflow chunks (general one-hot masks) for high-degree dsts.
- Device (SPMD): per-edge source rows fetched with batched dma_gather
  (GPSIMD gen time saturates ~19.4us per instruction, so layers 1/3 gather
  5 groups at once). Segment softmax via expv-scaled identity/overflow
  masks + TensorE matmuls; denominators via free-dim reduce + tiny overflow
  matmuls on dst partitions; normalization folded in after projection.
- Layer 1 aggregates raw x in transposed form ([feat, dst] PSUM) so the
  per-head W1 projection needs no transposes. Layers 2/3 aggregate
  [dst, feat] with mask as lhsT. Two chunked AllGathers exchange x2; one
  exchanges x3 rows between layers.
"""

import os
import numpy as np
import ml_dtypes

import concourse.bacc as bacc
import concourse.bass as bass
import concourse.mybir as mybir
import concourse.tile as tile
from concourse.bass_utils import run_bass_kernel_spmd
from concourse.masks import make_identity

P = 128
NC = 8
N = 10000
F_IN = 50
HID = 128
HEADS = 8
D = 1024
N_CLS = 121
NEG = 0.2
NODES = N // NC              # 1250
G = (NODES + P - 1) // P     # 10
LAST_ROWS = NODES - (G - 1) * P  # 98
GB = 5                       # groups per gather batch (layers 1 and 3)
SPLIT = GB * P               # 640: xe2 AllGather chunk split point
ROW1 = 128                   # [x(50) | pad | ssrc1 f32 slots 32..40 | pad]
S1F = 32
ROW2 = 1152                  # [x2(1024) | ssrc2 f32 slot 512 | pad]
S2F = 512
ROW3 = 128                   # [h3(121) | pad | ssrc3 f32 slot 61 | pad]
S3F = 61

BF16 = mybir.dt.bfloat16
F32 = mybir.dt.float32
I16 = mybir.dt.int16

_CACHE = {}
LAST_EXEC_NS = None


def _ap(x, dims):
    return bass.AP(x.tensor, x.offset, dims)


def build(nc, K1, K2, stop_after=None):
    K = K1 + K2
    # ---------------- external inputs ----------------
    x_base = nc.dram_tensor("x_base", [N, ROW1], BF16, kind="ExternalInput")
    x_t = nc.dram_tensor("x_t", [64, N], BF16, kind="ExternalInput")
    xtc = nc.dram_tensor("xtc", [64, G * P], BF16, kind="ExternalInput")
    v1 = nc.dram_tensor("v1", [64, 16], BF16, kind="ExternalInput")
    w1p = nc.dram_tensor("w1p", [64, D], BF16, kind="ExternalInput")
    w2 = nc.dram_tensor("w2", [D, D], BF16, kind="ExternalInput")
    w2s = nc.dram_tensor("w2s", [D, 2], BF16, kind="ExternalInput")
    w3e = nc.dram_tensor("w3e", [D, N_CLS + 2], BF16, kind="ExternalInput")
    brow = nc.dram_tensor("brow", [1, 2 * D + N_CLS], F32, kind="ExternalInput")
    idx1 = nc.dram_tensor("idx1", [P, G * K * 8], I16, kind="ExternalInput")
    idx2 = nc.dram_tensor("idx2", [P, G * K * 8], I16, kind="ExternalInput")
    idx3 = nc.dram_tensor("idx3", [P, G * K * 8], I16, kind="ExternalInput")
    dlo = nc.dram_tensor("dlo", [P, max(G * K2, 1)], F32, kind="ExternalInput")
    vb = nc.dram_tensor("vb", [P, G * K1], BF16, kind="ExternalInput")

    if stop_after == "a1":
        dbg_xb = nc.dram_tensor("dbg_xb", [N, ROW1], BF16, kind="ExternalOutput")
        dbg_sd = nc.dram_tensor("dbg_sd", [P, G * 8], F32, kind="ExternalOutput")
    elif stop_after == "b1":
        dbg_x2 = nc.dram_tensor("dbg_x2", [NODES, ROW2], BF16, kind="ExternalOutput")
        dbg_sd = nc.dram_tensor("dbg_sd", [P, G], F32, kind="ExternalOutput")
    elif stop_after == "b2":
        dbg_x3 = nc.dram_tensor("dbg_x3", [NODES, ROW3], BF16, kind="ExternalOutput")
        dbg_sd = nc.dram_tensor("dbg_sd", [P, G], F32, kind="ExternalOutput")
    else:
        out_shard = nc.dram_tensor("out_shard", [NODES, N_CLS], F32, kind="ExternalOutput")

    rg = [list(range(NC))]

    with tile.TileContext(nc) as tc:
        with (
            tc.tile_pool(name="const", bufs=1) as cst,
            tc.tile_pool(name="dram", bufs=1, space="DRAM") as dram,
        ):
            # ---------------- internal DRAM ----------------
            xe1 = dram.tile([N, ROW1], BF16)
            x2sh = dram.tile([NODES, ROW2], BF16)
            xe2 = dram.tile([N, ROW2], BF16, addr_space="Shared")
            x3sh = dram.tile([NODES, ROW3], BF16)
            xe3 = dram.tile([N, ROW3], BF16, addr_space="Shared")

            # ---------------- constants ----------------
            ident = cst.tile([P, P], F32)
            make_identity(nc, ident[:])
            identb = cst.tile([P, P], BF16)
            nc.vector.tensor_copy(identb[:], ident[:])
            iota_i = cst.tile([P, P], mybir.dt.int32)
            nc.gpsimd.iota(iota_i[:], pattern=[[1, P]], base=0, channel_multiplier=0)
            iota_b = cst.tile([P, P], BF16)
            nc.vector.tensor_copy(iota_b[:], iota_i[:])
            v1_sb = cst.tile([64, 16], BF16)
            nc.sync.dma_start(v1_sb[:], v1[:])
            w1_sb = cst.tile([64, D], BF16)
            nc.sync.dma_start(w1_sb[:], w1p[:])
            w2_sb = cst.tile([P, 8 * D], BF16)
            nc.sync.dma_start(w2_sb[:].rearrange("p (j n) -> p j n", j=8),
                              w2[:].rearrange("(j p) n -> p j n", p=P))
            w2s_sb = cst.tile([P, 8 * 2], BF16)
            nc.sync.dma_start(w2s_sb[:].rearrange("p (j n) -> p j n", j=8),
                              w2s[:].rearrange("(j p) n -> p j n", p=P))
            w3e_sb = cst.tile([P, 8 * (N_CLS + 2)], BF16)
            nc.sync.dma_start(w3e_sb[:].rearrange("p (j n) -> p j n", j=8),
                              w3e[:].rearrange("(j p) n -> p j n", p=P))
            # bias rows -> replicate across partitions by doubling sb2sb DMAs
            BW = 2 * D + N_CLS
            b_sb = cst.tile([P, BW], F32)
            nc.sync.dma_start(b_sb[0:1, :], brow[:])
            p = 1
            while p < P:
                nc.sync.dma_start(b_sb[p:2 * p, :], b_sb[0:p, :])
                p *= 2
            b1_ap = b_sb[:, 0:D]
            b2_ap = b_sb[:, D:2 * D]
            b3_ap = b_sb[:, 2 * D:2 * D + N_CLS]

            idx1_sb = cst.tile([P, G * K * 8], I16)
            nc.sync.dma_start(idx1_sb[:], idx1[:])
            idx2_sb = cst.tile([P, G * K * 8], I16)
            nc.sync.dma_start(idx2_sb[:], idx2[:])
            idx3_sb = cst.tile([P, G * K * 8], I16)
            nc.sync.dma_start(idx3_sb[:], idx3[:])
            dlo_sb = cst.tile([P, max(G * K2, 1)], F32)
            nc.sync.dma_start(dlo_sb[:], dlo[:])
            vb_sb = cst.tile([P, G * K1], BF16)
            nc.sync.dma_start(vb_sb[:], vb[:])

            sdg1 = cst.tile([P, G * 8], F32)
            nc.vector.memset(sdg1[:], 0.0)
            sd2_sb = cst.tile([P, G], F32)
            nc.vector.memset(sd2_sb[:], 0.0)
            sd3_sb = cst.tile([P, G], F32)
            nc.vector.memset(sd3_sb[:], 0.0)

            # =============== Phase A1: s_src1 into x_base; own-shard s_dst1 ===============
            with (
                tc.tile_pool(name="a1", bufs=3) as a1,
                tc.tile_pool(name="a1ps", bufs=2, space="PSUM") as a1ps,
            ):
                xt_sb = a1.tile([64, N], BF16, tag="xt")
                nc.sync.dma_start(xt_sb[:], x_t[:])
                xtc_sb = a1.tile([64, G * P], BF16, tag="xtc")
                nc.sync.dma_start(xtc_sb[:], xtc[:])
                # own-shard s_dst1
                for g in range(G):
                    rows = P if g < G - 1 else LAST_ROWS
                    psd = a1ps.tile([P, 8], F32, tag="psd", space="PSUM")
                    nc.tensor.matmul(psd[:rows], lhsT=xtc_sb[:, g * P:g * P + rows],
                                     rhs=v1_sb[:, 8:16], start=True, stop=True)
                    nc.scalar.copy(sdg1[:rows, g * 8:(g + 1) * 8], psd[:rows])
                # xe1 rows = x_base rows with s_src1 inserted into f32 slots
                ntiles = (N + P - 1) // P   # 79
                BT = 8
                for t0 in range(0, ntiles, BT):
                    nb = min(BT, ntiles - t0)
                    r0 = t0 * P
                    rtot = min(BT * P, N - r0)
                    pss = a1ps.tile([P, BT * 8], F32, tag="pss", space="PSUM")
                    rt = a1.tile([P, BT * ROW1], BF16, tag="rt")
                    if rtot == BT * P:
                        nc.sync.dma_start(
                            rt[:].rearrange("p (q w) -> p q w", q=BT),
                            x_base[r0:r0 + rtot, :].rearrange("(q p) w -> p q w", p=P))
                    else:
                        for q in range(nb):
                            rr = min(P, N - (t0 + q) * P)
                            nc.sync.dma_start(rt[:rr, q * ROW1:q * ROW1 + ROW1],
                                              x_base[(t0 + q) * P:(t0 + q) * P + rr, :])
                    for q in range(nb):
                        rr = min(P, N - (t0 + q) * P)
                        nc.tensor.matmul(pss[:rr, q * 8:(q + 1) * 8],
                                         lhsT=xt_sb[:, (t0 + q) * P:(t0 + q) * P + rr],
                                         rhs=v1_sb[:, 0:8], start=True, stop=True)
                    rtf = rt[:].bitcast(F32)  # [P, BT*64]
                    nc.vector.tensor_copy(
                        bass.AP(rtf.tensor, rtf.offset + S1F, [rtf.ap[0], [64, nb], [1, 8]]),
                        bass.AP(pss[:].tensor, pss[:].offset, [pss[:].ap[0], [8, nb], [1, 8]]))
                    if rtot == BT * P:
                        nc.sync.dma_start(
                            xe1[r0:r0 + rtot, :].rearrange("(q p) w -> p q w", p=P),
                            rt[:].rearrange("p (q w) -> p q w", q=BT))
                    else:
                        for q in range(nb):
                            rr = min(P, N - (t0 + q) * P)
                            nc.sync.dma_start(xe1[(t0 + q) * P:(t0 + q) * P + rr, :],
                                              rt[:rr, q * ROW1:q * ROW1 + ROW1])

            if stop_after == "a1":
                nc.sync.dma_start(dbg_xb[:], xe1[:])
                nc.sync.dma_start(dbg_sd[:], sdg1[:])
                return nc

            # =============== Phase B1: layer 1 ===============
            with (
                tc.tile_pool(name="b1", bufs=2) as b1p,
                tc.tile_pool(name="b1w", bufs=2) as b1w,
                tc.tile_pool(name="b1psA", bufs=1, space="PSUM") as psA,
                tc.tile_pool(name="b1psT", bufs=2, space="PSUM") as psTp,
                tc.tile_pool(name="b1psB", bufs=1, space="PSUM") as psB,
            ):
                for bt in range(G // GB):
                    gtile = b1p.tile([P, GB * K * ROW1], BF16, tag="g1")
                    nc.gpsimd.dma_gather(
                        out_ap=gtile[:].rearrange("p (k w) -> p k w", k=GB * K),
                        in_ap=xe1[:],
                        idxs_ap=idx1_sb[:, bt * GB * K * 8:(bt + 1) * GB * K * 8],
                        num_idxs=GB * K * P, num_idxs_reg=GB * K * P,
                        elem_size=ROW1, single_packet=False)
                    for gi in range(GB):
                        g = bt * GB + gi
                        rows = P if g < G - 1 else LAST_ROWS
                        gv = gtile[:, gi * K * ROW1:(gi + 1) * K * ROW1]
                        gf = gv.bitcast(F32)  # [P, K*64]
                        sdgv = sdg1[:, g * 8:(g + 1) * 8]
                        # ---- alpha [P, K*8] (col = k*8+h) ----
                        alpha = b1w.tile([P, K * 8], F32, tag="alpha")
                        # identity slots, iterated (h, k)
                        nc.vector.tensor_tensor(
                            out=_ap(alpha[:, 0:K1 * 8], [alpha[:].ap[0], [1, 8], [8, K1]]),
                            in0=bass.AP(gf.tensor, gf.offset + S1F, [gf.ap[0], [1, 8], [64, K1]]),
                            in1=bass.AP(sdgv.tensor, sdgv.offset, [sdgv.ap[0], [1, 8], [0, K1]]),
                            op=mybir.AluOpType.add)
                        # overflow one-hot masks + transposed copies
                        if K2 > 0:
                            m01 = b1w.tile([P, K2 * P], BF16, tag="m01")
                            for j in range(K2):
                                nc.vector.tensor_scalar(
                                    out=m01[:, j * P:(j + 1) * P], in0=iota_b[:],
                                    scalar1=dlo_sb[:, g * K2 + j:g * K2 + j + 1],
                                    scalar2=None, op0=mybir.AluOpType.is_equal)
                            m01T = b1w.tile([P, K2 * P], F32, tag="m01T")
                            ps_sd = psA.tile([P, K2 * 8 + 8], F32, tag="ps_sd", space="PSUM")
                            for j in range(K2):
                                pst = psTp.tile([P, P], BF16, tag="psT", space="PSUM")
                                nc.tensor.transpose(out=pst[:], in_=m01[:, j * P:(j + 1) * P],
                                                    identity=identb[:])
                                nc.scalar.copy(m01T[:, j * P:(j + 1) * P], pst[:])
                                nc.tensor.matmul(ps_sd[:, j * 8:(j + 1) * 8],
                                                 lhsT=m01T[:, j * P:(j + 1) * P],
                                                 rhs=sdgv, start=True, stop=True)
                            # overflow alpha: layout (k2, h)
                            nc.vector.tensor_tensor(
                                out=_ap(alpha[:, K1 * 8:K * 8],
                                        [alpha[:].ap[0], [8, K2], [1, 8]]),
                                in0=bass.AP(gf.tensor, gf.offset + K1 * 64 + S1F,
                                            [gf.ap[0], [64, K2], [1, 8]]),
                                in1=ps_sd[:, 0:K2 * 8].rearrange("p (k h) -> p k h", k=K2),
                                op=mybir.AluOpType.add)
                        # ---- exp(leaky_relu(alpha)) (+ validity)
                        lr = b1w.tile([P, K * 8], F32, tag="lr")
                        nc.vector.tensor_scalar_mul(lr[:], alpha[:], NEG)
                        nc.vector.tensor_tensor(out=lr[:], in0=alpha[:], in1=lr[:],
                                                op=mybir.AluOpType.max)
                        ex = b1w.tile([P, K * 8], BF16, tag="ex")
                        nc.scalar.activation(ex[:], lr[:], mybir.ActivationFunctionType.Exp)
                        vbv = vb_sb[:, g * K1:(g + 1) * K1]
                        nc.vector.tensor_tensor(
                            out=_ap(ex[:, 0:K1 * 8], [ex[:].ap[0], [1, 8], [8, K1]]),
                            in0=_ap(ex[:, 0:K1 * 8], [ex[:].ap[0], [1, 8], [8, K1]]),
                            in1=bass.AP(vbv.tensor, vbv.offset, [vbv.ap[0], [0, 8], [1, K1]]),
                            op=mybir.AluOpType.mult)
                        # ---- denominators on dst partitions
                        den = b1w.tile([P, 8], F32, tag="den")
                        nc.vector.tensor_reduce(
                            out=den[:].rearrange("p (h o) -> p h o", h=8),
                            in_=_ap(ex[:, 0:K1 * 8], [ex[:].ap[0], [1, 8], [8, K1]]),
                            axis=mybir.AxisListType.X, op=mybir.AluOpType.add)
                        if K2 > 0:
                            psden = ps_sd[:, K2 * 8:K2 * 8 + 8]
                            for j in range(K2):
                                nc.tensor.matmul(
                                    psden, lhsT=m01[:, j * P:(j + 1) * P],
                                    rhs=ex[:, K1 * 8 + j * 8:K1 * 8 + (j + 1) * 8],
                                    start=(j == 0), stop=(j == K2 - 1))
                            nc.vector.scalar_tensor_tensor(
                                out=den[:], in0=den[:], scalar=1e-30,
                                op0=mybir.AluOpType.max, in1=psden,
                                op1=mybir.AluOpType.add)
                        else:
                            nc.vector.tensor_scalar_max(den[:], den[:], 1e-30)
                        rec = b1w.tile([P, 8], F32, tag="rec")
                        nc.vector.reciprocal(rec[:], den[:])
                        # ---- masks [P, K*1024] (k, h, d)
                        mask = b1w.tile([P, K * 1024], BF16, tag="mask")
                        nc.vector.tensor_tensor(
                            out=mask[:, 0:K1 * 1024].rearrange(
                                "p (k h d) -> p k h d", k=K1, h=8),
                            in0=_ap(identb[:], [identb[:].ap[0], [0, K1], [0, 8], [1, P]]),
                            in1=_ap(ex[:, 0:K1 * 8],
                                    [ex[:].ap[0], [8, K1], [1, 8], [0, P]]),
                            op=mybir.AluOpType.mult)
                        if K2 > 0:
                            nc.vector.tensor_tensor(
                                out=mask[:, K1 * 1024:].rearrange(
                                    "p (k h d) -> p k h d", k=K2, h=8),
                                in0=_ap(m01[:], [m01[:].ap[0], [P, K2], [0, 8], [1, P]]),
                                in1=_ap(ex[:, K1 * 8:],
                                        [ex[:].ap[0], [8, K2], [1, 8], [0, P]]),
                                op=mybir.AluOpType.mult)
                        # ---- aggregation (transposed): pT[feat, (h,d)]
                        pT0 = psB.tile([64, 512], F32, tag="pT0", space="PSUM")
                        pT1 = psB.tile([64, 512], F32, tag="pT1", space="PSUM")
                        for k in range(K):
                            st, sp = (k == 0), (k == K - 1)
                            nc.tensor.matmul(pT0[:F_IN, :],
                                             lhsT=gv[:, k * ROW1:k * ROW1 + F_IN],
                                             rhs=mask[:, k * 1024:k * 1024 + 512],
                                             start=st, stop=sp)
                            nc.tensor.matmul(pT1[:F_IN, :],
                                             lhsT=gv[:, k * ROW1:k * ROW1 + F_IN],
                                             rhs=mask[:, k * 1024 + 512:(k + 1) * 1024],
                                             start=st, stop=sp)
                        aggn = b1w.tile([64, 1024], BF16, tag="aggn")
                        nc.scalar.copy(aggn[:F_IN, 0:512], pT0[:F_IN, :])
                        nc.scalar.copy(aggn[:F_IN, 512:1024], pT1[:F_IN, :])
                        # ---- projection per head
                        pxa = psB.tile([P, 512], F32, tag="pxa", space="PSUM")
                        pxb = psB.tile([P, 512], F32, tag="pxb", space="PSUM")
                        for h in range(HEADS):
                            px = pxa if h < 4 else pxb
                            nc.tensor.matmul(px[:, (h % 4) * HID:(h % 4 + 1) * HID],
                                             lhsT=aggn[:F_IN, h * HID:(h + 1) * HID],
                                             rhs=w1_sb[:F_IN, h * HID:(h + 1) * HID],
                                             start=True, stop=True)
                        # ---- x2 = elu(px * rec + b1)
                        rt2 = b1w.tile([P, ROW2], BF16, tag="rt2")
                        for half in range(2):
                            px = pxa if half == 0 else pxb
                            cl = slice(half * 512, half * 512 + 512)
                            recv = rec[:, half * 4:half * 4 + 4]
                            y = b1w.tile([P, 512], F32, tag="y")
                            nc.vector.tensor_tensor(
                                out=y[:].rearrange("p (h d) -> p h d", h=4),
                                in0=px[:].rearrange("p (h d) -> p h d", h=4),
                                in1=bass.AP(recv.tensor, recv.offset,
                                            [recv.ap[0], [1, 4], [0, HID]]),
                                op=mybir.AluOpType.mult)
                            yb = b1w.tile([P, 512], BF16, tag="yb")
                            nc.vector.tensor_tensor(out=yb[:], in0=y[:], in1=b1_ap[:, cl],
                                                    op=mybir.AluOpType.add)
                            e = b1w.tile([P, 512], BF16, tag="e")
                            nc.scalar.activation(e[:], yb[:], mybir.ActivationFunctionType.Exp)
                            nc.vector.tensor_scalar(out=e[:], in0=e[:], scalar1=1.0,
                                                    scalar2=-1.0, op0=mybir.AluOpType.min,
                                                    op1=mybir.AluOpType.add)
                            r = b1w.tile([P, 512], BF16, tag="r")
                            nc.scalar.activation(r[:], yb[:], mybir.ActivationFunctionType.Relu)
                            nc.vector.tensor_tensor(out=rt2[:, cl], in0=r[:], in1=e[:],
                                                    op=mybir.AluOpType.add)
                        # ---- s_src2 / s_dst2 via x2T
                        x2t = b1w.tile([P, 8 * P], BF16, tag="x2t")
                        for j in range(8):
                            pst = psTp.tile([P, P], BF16, tag="psT", space="PSUM")
                            nc.tensor.transpose(out=pst[:], in_=rt2[:, j * P:(j + 1) * P],
                                                identity=identb[:])
                            nc.scalar.copy(x2t[:, j * P:(j + 1) * P], pst[:])
                        ps2 = psA.tile([P, 2], F32, tag="ps2", space="PSUM")
                        for j in range(8):
                            nc.tensor.matmul(ps2[:], lhsT=x2t[:, j * P:(j + 1) * P],
                                             rhs=w2s_sb[:, j * 2:(j + 1) * 2],
                                             start=(j == 0), stop=(j == 7))
                        rt2f = rt2[:].bitcast(F32)
                        nc.vector.tensor_copy(rt2f[:, S2F:S2F + 1], ps2[:, 0:1])
                        nc.vector.tensor_copy(sd2_sb[:, g:g + 1], ps2[:, 1:2])
                        nc.sync.dma_start(x2sh[g * P:g * P + rows, :], rt2[:rows])

            if stop_after == "b1":
                nc.sync.dma_start(dbg_x2[:], x2sh[:])
                nc.sync.dma_start(dbg_sd[:], sd2_sb[:])
                return nc

            nc.gpsimd.collective_compute("AllGather", mybir.AluOpType.bypass,
                                         replica_groups=rg, ins=[x2sh[:]],
                                         outs=[xe2[:]])

            # =============== Phase B2: layer 2 ===============
            with (
                tc.tile_pool(name="b2", bufs=2) as b2p,
                tc.tile_pool(name="b2w", bufs=2) as b2w,
                tc.tile_pool(name="b2psA", bufs=1, space="PSUM") as psA2,
                tc.tile_pool(name="b2psT", bufs=2, space="PSUM") as psTp2,
                tc.tile_pool(name="b2psB", bufs=1, space="PSUM") as psB2,
            ):
                for g in range(G):
                    rows = P if g < G - 1 else LAST_ROWS
                    gtile = b2p.tile([P, K * ROW2], BF16, tag="g2")
                    nc.gpsimd.dma_gather(
                        out_ap=gtile[:].rearrange("p (k w) -> p k w", k=K),
                        in_ap=xe2[:], idxs_ap=idx2_sb[:, g * K * 8:(g + 1) * K * 8],
                        num_idxs=K * P, num_idxs_reg=K * P, elem_size=ROW2,
                        single_packet=False)
                    gf = gtile[:].bitcast(F32)  # [P, K*576]
                    sdv = sd2_sb[:, g:g + 1]
                    alpha = b2w.tile([P, K], F32, tag="alpha")
                    nc.vector.tensor_tensor(
                        out=alpha[:, 0:K1],
                        in0=bass.AP(gf.tensor, gf.offset + S2F, [gf.ap[0], [576, K1]]),
                        in1=bass.AP(sdv.tensor, sdv.offset, [sdv.ap[0], [0, K1]]),
                        op=mybir.AluOpType.add)
                    if K2 > 0:
                        m01 = b2w.tile([P, K2 * P], BF16, tag="m01")
                        for j in range(K2):
                            nc.vector.tensor_scalar(
                                out=m01[:, j * P:(j + 1) * P], in0=iota_b[:],
                                scalar1=dlo_sb[:, g * K2 + j:g * K2 + j + 1],
                                scalar2=None, op0=mybir.AluOpType.is_equal)
                        m01T = b2w.tile([P, K2 * P], F32, tag="m01T")
                        ps_sd = psA2.tile([P, K2 + 1], F32, tag="ps_sd", space="PSUM")
                        for j in range(K2):
                            pst = psTp2.tile([P, P], BF16, tag="psT", space="PSUM")
                            nc.tensor.transpose(out=pst[:], in_=m01[:, j * P:(j + 1) * P],
                                                identity=identb[:])
                            nc.scalar.copy(m01T[:, j * P:(j + 1) * P], pst[:])
                            nc.tensor.matmul(ps_sd[:, j:j + 1],
                                             lhsT=m01T[:, j * P:(j + 1) * P],
                                             rhs=sdv, start=True, stop=True)
                        nc.vector.tensor_tensor(
                            out=alpha[:, K1:K],
                            in0=bass.AP(gf.tensor, gf.offset + K1 * 576 + S2F,
                                        [gf.ap[0], [576, K2]]),
                            in1=ps_sd[:, 0:K2], op=mybir.AluOpType.add)
                    lr = b2w.tile([P, K], F32, tag="lr")
                    nc.vector.tensor_scalar_mul(lr[:], alpha[:], NEG)
                    nc.vector.tensor_tensor(out=lr[:], in0=alpha[:], in1=lr[:],
                                            op=mybir.AluOpType.max)
                    ex = b2w.tile([P, K], BF16, tag="ex")
                    nc.scalar.activation(ex[:], lr[:], mybir.ActivationFunctionType.Exp)
                    vbv = vb_sb[:, g * K1:(g + 1) * K1]
                    nc.vector.tensor_tensor(out=ex[:, 0:K1], in0=ex[:, 0:K1],
                                            in1=vbv, op=mybir.AluOpType.mult)
                    den = b2w.tile([P, 1], F32, tag="den")
                    nc.vector.tensor_reduce(out=den[:], in_=ex[:, 0:K1],
                                            axis=mybir.AxisListType.X,
                                            op=mybir.AluOpType.add)
                    if K2 > 0:
                        psden = ps_sd[:, K2:K2 + 1]
                        for j in range(K2):
                            nc.tensor.matmul(psden, lhsT=m01[:, j * P:(j + 1) * P],
                                             rhs=ex[:, K1 + j:K1 + j + 1],
                                             start=(j == 0), stop=(j == K2 - 1))
                        nc.vector.scalar_tensor_tensor(
                            out=den[:], in0=den[:], scalar=1e-30,
                            op0=mybir.AluOpType.max, in1=psden,
                            op1=mybir.AluOpType.add)
                    else:
                        nc.vector.tensor_scalar_max(den[:], den[:], 1e-30)
                    rec = b2w.tile([P, 1], F32, tag="rec")
                    nc.vector.reciprocal(rec[:], den[:])
                    # masks (k, d)
                    mask = b2w.tile([P, K * P], BF16, tag="mask")
                    nc.vector.tensor_tensor(
                        out=mask[:, 0:K1 * P].rearrange("p (k d) -> p k d", k=K1),
                        in0=_ap(identb[:], [identb[:].ap[0], [0, K1], [1, P]]),
                        in1=_ap(ex[:, 0:K1], [ex[:].ap[0], [1, K1], [0, P]]),
                        op=mybir.AluOpType.mult)
                    if K2 > 0:
                        nc.vector.tensor_tensor(
                            out=mask[:, K1 * P:].rearrange("p (k d) -> p k d", k=K2),
                            in0=m01[:].rearrange("p (k d) -> p k d", k=K2),
                            in1=_ap(ex[:, K1:K], [ex[:].ap[0], [1, K2], [0, P]]),
                            op=mybir.AluOpType.mult)
                    # aggregation [dst, feat]
                    psa = psB2.tile([P, 512], F32, tag="psa", space="PSUM")
                    psb = psB2.tile([P, 512], F32, tag="psb", space="PSUM")
                    for k in range(K):
                        st, sp = (k == 0), (k == K - 1)
                        nc.tensor.matmul(psa[:], lhsT=mask[:, k * P:(k + 1) * P],
                                         rhs=gtile[:, k * ROW2:k * ROW2 + 512],
                                         start=st, stop=sp)
                        nc.tensor.matmul(psb[:], lhsT=mask[:, k * P:(k + 1) * P],
                                         rhs=gtile[:, k * ROW2 + 512:k * ROW2 + 1024],
                                         start=st, stop=sp)
                    agg2 = b2w.tile([P, D], BF16, tag="agg2")
                    nc.scalar.mul(agg2[:, 0:512], psa[:], rec[:, 0:1])
                    nc.scalar.mul(agg2[:, 512:1024], psb[:], rec[:, 0:1])
                    a2t = b2w.tile([P, 8 * P], BF16, tag="a2t")
                    for j in range(8):
                        pst = psTp2.tile([P, P], BF16, tag="psT", space="PSUM")
                        nc.tensor.transpose(out=pst[:], in_=agg2[:, j * P:(j + 1) * P],
                                            identity=identb[:])
                        nc.scalar.copy(a2t[:, j * P:(j + 1) * P], pst[:])
                    pso_a = psB2.tile([P, 512], F32, tag="pso_a", space="PSUM")
                    pso_b = psB2.tile([P, 512], F32, tag="pso_b", space="PSUM")
                    for j in range(8):
                        nc.tensor.matmul(pso_a[:], lhsT=a2t[:, j * P:(j + 1) * P],
                                         rhs=w2_sb[:, j * D:j * D + 512],
                                         start=(j == 0), stop=(j == 7))
                        nc.tensor.matmul(pso_b[:], lhsT=a2t[:, j * P:(j + 1) * P],
                                         rhs=w2_sb[:, j * D + 512:(j + 1) * D],
                                         start=(j == 0), stop=(j == 7))
                    x3 = b2w.tile([P, D], BF16, tag="x3")
                    for half in range(2):
                        pso = pso_a if half == 0 else pso_b
                        cl = slice(half * 512, half * 512 + 512)
                        yb = b2w.tile([P, 512], BF16, tag="yb")
                        nc.vector.tensor_tensor(out=yb[:], in0=pso[:], in1=b2_ap[:, cl],
                                                op=mybir.AluOpType.add)
                        e = b2w.tile([P, 512], BF16, tag="e")
                        nc.scalar.activation(e[:], yb[:], mybir.ActivationFunctionType.Exp)
                        nc.vector.tensor_scalar(out=e[:], in0=e[:], scalar1=1.0,
                                                scalar2=-1.0, op0=mybir.AluOpType.min,
                                                op1=mybir.AluOpType.add)
                        r = b2w.tile([P, 512], BF16, tag="r")
                        nc.scalar.activation(r[:], yb[:], mybir.ActivationFunctionType.Relu)
                        nc.vector.tensor_tensor(out=x3[:, cl], in0=r[:], in1=e[:],
                                                op=mybir.AluOpType.add)
                    x3t = b2w.tile([P, 8 * P], BF16, tag="x3t")
                    for j in range(8):
                        pst = psTp2.tile([P, P], BF16, tag="psT", space="PSUM")
                        nc.tensor.transpose(out=pst[:], in_=x3[:, j * P:(j + 1) * P],
                                            identity=identb[:])
                        nc.scalar.copy(x3t[:, j * P:(j + 1) * P], pst[:])
                    ps3 = psA2.tile([P, N_CLS + 2], F32, tag="ps3", space="PSUM")
                    for j in range(8):
                        nc.tensor.matmul(ps3[:], lhsT=x3t[:, j * P:(j + 1) * P],
                                         rhs=w3e_sb[:, j * (N_CLS + 2):(j + 1) * (N_CLS + 2)],
                                         start=(j == 0), stop=(j == 7))
                    rt3 = b2w.tile([P, ROW3], BF16, tag="rt3")
                    nc.vector.tensor_copy(rt3[:, 0:N_CLS], ps3[:, 0:N_CLS])
                    nc.vector.memset(rt3[:, N_CLS:ROW3], 0.0)
                    rt3f = rt3[:].bitcast(F32)
                    nc.vector.tensor_copy(rt3f[:, S3F:S3F + 1], ps3[:, N_CLS:N_CLS + 1])
                    nc.vector.tensor_copy(sd3_sb[:, g:g + 1], ps3[:, N_CLS + 1:N_CLS + 2])
                    nc.sync.dma_start(x3sh[g * P:g * P + rows, :], rt3[:rows])

            if stop_after == "b2":
                nc.sync.dma_start(dbg_x3[:], x3sh[:])
                nc.sync.dma_start(dbg_sd[:], sd3_sb[:])
                return nc

            nc.gpsimd.collective_compute("AllGather", mybir.AluOpType.bypass,
                                         replica_groups=rg, ins=[x3sh[:]],
                                         outs=[xe3[:]])

            # =============== Phase B3: layer 3 ===============
            with (
                tc.tile_pool(name="b3", bufs=2) as b3p,
                tc.tile_pool(name="b3w", bufs=2) as b3w,
                tc.tile_pool(name="b3ps", bufs=1, space="PSUM") as psA3,
                tc.tile_pool(name="b3psT", bufs=2, space="PSUM") as psTp3,
            ):
                for bt in range(G // GB):
                    gtile = b3p.tile([P, GB * K * ROW3], BF16, tag="g3")
                    nc.gpsimd.dma_gather(
                        out_ap=gtile[:].rearrange("p (k w) -> p k w", k=GB * K),
                        in_ap=xe3[:],
                        idxs_ap=idx3_sb[:, bt * GB * K * 8:(bt + 1) * GB * K * 8],
                        num_idxs=GB * K * P, num_idxs_reg=GB * K * P,
                        elem_size=ROW3, single_packet=False)
                    for gi in range(GB):
                        g = bt * GB + gi
                        rows = P if g < G - 1 else LAST_ROWS
                        gv = gtile[:, gi * K * ROW3:(gi + 1) * K * ROW3]
                        gf = gv.bitcast(F32)
                        sdv = sd3_sb[:, g:g + 1]
                        alpha = b3w.tile([P, K], F32, tag="alpha")
                        nc.vector.tensor_tensor(
                            out=alpha[:, 0:K1],
                            in0=bass.AP(gf.tensor, gf.offset + S3F, [gf.ap[0], [64, K1]]),
                            in1=bass.AP(sdv.tensor, sdv.offset, [sdv.ap[0], [0, K1]]),
                            op=mybir.AluOpType.add)
                        if K2 > 0:
                            m01 = b3w.tile([P, K2 * P], BF16, tag="m01")
                            for j in range(K2):
                                nc.vector.tensor_scalar(
                                    out=m01[:, j * P:(j + 1) * P], in0=iota_b[:],
                                    scalar1=dlo_sb[:, g * K2 + j:g * K2 + j + 1],
                                    scalar2=None, op0=mybir.AluOpType.is_equal)
                            m01T = b3w.tile([P, K2 * P], F32, tag="m01T")
                            ps_sd = psA3.tile([P, K2 + 1], F32, tag="ps_sd", space="PSUM")
                            for j in range(K2):
                                pst = psTp3.tile([P, P], BF16, tag="psT", space="PSUM")
                                nc.tensor.transpose(out=pst[:],
                                                    in_=m01[:, j * P:(j + 1) * P],
                                                    identity=identb[:])
                                nc.scalar.copy(m01T[:, j * P:(j + 1) * P], pst[:])
                                nc.tensor.matmul(ps_sd[:, j:j + 1],
                                                 lhsT=m01T[:, j * P:(j + 1) * P],
                                                 rhs=sdv, start=True, stop=True)
                            nc.vector.tensor_tensor(
                                out=alpha[:, K1:K],
                                in0=bass.AP(gf.tensor, gf.offset + K1 * 64 + S3F,
                                            [gf.ap[0], [64, K2]]),
                                in1=ps_sd[:, 0:K2], op=mybir.AluOpType.add)
                        lr = b3w.tile([P, K], F32, tag="lr")
                        nc.vector.tensor_scalar_mul(lr[:], alpha[:], NEG)
                        nc.vector.tensor_tensor(out=lr[:], in0=alpha[:], in1=lr[:],
                                                op=mybir.AluOpType.max)
                        ex = b3w.tile([P, K], BF16, tag="ex")
                        nc.scalar.activation(ex[:], lr[:],
                                             mybir.ActivationFunctionType.Exp)
                        vbv = vb_sb[:, g * K1:(g + 1) * K1]
                        nc.vector.tensor_tensor(out=ex[:, 0:K1], in0=ex[:, 0:K1],
                                                in1=vbv, op=mybir.AluOpType.mult)
                        den = b3w.tile([P, 1], F32, tag="den")
                        nc.vector.tensor_reduce(out=den[:], in_=ex[:, 0:K1],
                                                axis=mybir.AxisListType.X,
                                                op=mybir.AluOpType.add)
                        if K2 > 0:
                            psden = ps_sd[:, K2:K2 + 1]
                            for j in range(K2):
                                nc.tensor.matmul(psden,
                                                 lhsT=m01[:, j * P:(j + 1) * P],
                                                 rhs=ex[:, K1 + j:K1 + j + 1],
                                                 start=(j == 0), stop=(j == K2 - 1))
                            nc.vector.scalar_tensor_tensor(
                                out=den[:], in0=den[:], scalar=1e-30,
                                op0=mybir.AluOpType.max, in1=psden,
                                op1=mybir.AluOpType.add)
                        else:
                            nc.vector.tensor_scalar_max(den[:], den[:], 1e-30)
                        rec = b3w.tile([P, 1], F32, tag="rec")
                        nc.vector.reciprocal(rec[:], den[:])
                        mask = b3w.tile([P, K * P], BF16, tag="mask")
                        nc.vector.tensor_tensor(
                            out=mask[:, 0:K1 * P].rearrange("p (k d) -> p k d", k=K1),
                            in0=_ap(identb[:], [identb[:].ap[0], [0, K1], [1, P]]),
                            in1=_ap(ex[:, 0:K1], [ex[:].ap[0], [1, K1], [0, P]]),
                            op=mybir.AluOpType.mult)
                        if K2 > 0:
                            nc.vector.tensor_tensor(
                                out=mask[:, K1 * P:].rearrange("p (k d) -> p k d", k=K2),
                                in0=m01[:].rearrange("p (k d) -> p k d", k=K2),
                                in1=_ap(ex[:, K1:K], [ex[:].ap[0], [1, K2], [0, P]]),
                                op=mybir.AluOpType.mult)
                        pso = psA3.tile([P, N_CLS], F32, tag="pso", space="PSUM")
                        for k in range(K):
                            nc.tensor.matmul(pso[:], lhsT=mask[:, k * P:(k + 1) * P],
                                             rhs=gv[:, k * ROW3:k * ROW3 + N_CLS],
                                             start=(k == 0), stop=(k == K - 1))
                        o = b3w.tile([P, N_CLS], F32, tag="o")
                        nc.scalar.mul(o[:], pso[:], rec[:, 0:1])
                        nc.vector.tensor_tensor(out=o[:], in0=o[:], in1=b3_ap,
                                                op=mybir.AluOpType.add)
                        nc.sync.dma_start(out_shard[g * P:g * P + rows, :], o[:rows])
    return nc


def _wrap_idx(idx_i16):
    n = idx_i16.shape[0]
    w = idx_i16.reshape(n // 16, 16).T
    return np.tile(w, (8, 1)).copy()


def _host_prep(edge_index):
    src = np.concatenate([edge_index[0], np.arange(N, dtype=np.int64)]).astype(np.int64)
    dst = np.concatenate([edge_index[1], np.arange(N, dtype=np.int64)]).astype(np.int64)
    order = np.argsort(dst, kind="stable")
    s, d = src[order], dst[order]
    starts = np.searchsorted(d, np.arange(N + 1))
    rank = np.arange(len(d)) - starts[d]
    deg = (starts[1:] - starts[:-1]).astype(np.int64)

    # pick K1 minimizing K1 + max overflow chunks
    group_of = np.minimum(np.arange(N) // NODES * G + (np.arange(N) % NODES) // P,
                          NC * G - 1)
    best = None
    for K1 in range(6, 36):
        ov = np.maximum(deg - K1, 0)
        ov_per_g = np.bincount(group_of, weights=ov, minlength=NC * G)
        K2 = int(-(-ov_per_g.max() // P))
        tot = K1 + K2
        if best is None or tot < best[0] or (tot == best[0] and K2 < best[2]):
            best = (tot, K1, K2)
    _, K1, K2 = best
    K = K1 + K2

    # identity slots
    idx_id = np.zeros((N, K1), np.int64)
    vb_id = np.zeros((N, K1), np.float32)
    selm = rank < K1
    idx_id[d[selm], rank[selm]] = s[selm]
    vb_id[d[selm], rank[selm]] = 1.0

    # per-(core,group) flat edge lists (slot order k*128+p) + overflow dl
    flat = np.zeros((NC, G, K * P), np.int64)
    dlo_arr = np.full((NC, G, max(K2, 1) * P), 128.0, np.float32)
    for c in range(NC):
        for g in range(G):
            lo = c * NODES + g * P
            hi = min(c * NODES + NODES, lo + P)
            rows = hi - lo
            blk = np.zeros((K1, P), np.int64)
            blk[:, :rows] = idx_id[lo:hi, :].T
            flat[c, g, :K1 * P] = blk.reshape(-1)
            if K2 > 0:
                e0, e1 = starts[lo], starts[hi]
                sel = rank[e0:e1] >= K1
                ovs = s[e0:e1][sel]
                ovd = d[e0:e1][sel] - lo
                nov = len(ovs)
                ov_idx = np.zeros(K2 * P, np.int64)
                ov_dl = np.full(K2 * P, 128.0, np.float32)
                ov_idx[:nov] = ovs
                ov_dl[:nov] = ovd
                flat[c, g, K1 * P:] = ov_idx
                # dlo layout [p, j] = ov_dl[j*128+p]
                dlo_arr[c, g] = ov_dl.reshape(K2, P).T.reshape(-1)

    idx1_in = np.zeros((NC, P, G * K * 8), np.int16)
    idx2_in = np.zeros((NC, P, G * K * 8), np.int16)
    idx3_in = np.zeros((NC, P, G * K * 8), np.int16)
    for c in range(NC):
        f1 = flat[c].reshape(-1)
        w1 = _wrap_idx(f1.astype(np.int16))
        w2 = _wrap_idx(f1.astype(np.int16))
        w3 = _wrap_idx(f1.astype(np.int16))
        idx1_in[c] = w1
        idx2_in[c] = w2
        idx3_in[c] = w3

    dlo_in = np.zeros((NC, P, max(G * K2, 1)), np.float32)
    vb_in = np.zeros((NC, P, G * K1), np.float32)
    for c in range(NC):
        for g in range(G):
            if K2 > 0:
                dlo_in[c, :, g * K2:(g + 1) * K2] = dlo_arr[c, g].reshape(P, K2)
            lo = c * NODES + g * P
            hi = min(c * NODES + NODES, lo + P)
            vb_in[c, :hi - lo, g * K1:(g + 1) * K1] = vb_id[lo:hi, :]
    return K1, K2, idx1_in, idx2_in, idx3_in, dlo_in, vb_in


def _prep_in_maps(inputs, K1, K2, idx1_in, idx2_in, idx3_in, dlo_in, vb_in):
    bf = lambda a: np.asarray(a, np.float32).astype(ml_dtypes.bfloat16)
    x = np.asarray(inputs["x"], np.float32)
    W1f = np.asarray(inputs["W1"], np.float32)
    a_s1 = np.asarray(inputs["a_src1"], np.float32)
    a_d1 = np.asarray(inputs["a_dst1"], np.float32)
    W1h = W1f.reshape(F_IN, HEADS, HID)
    V = np.zeros((64, 16), np.float32)
    V[:F_IN, 0:8] = np.einsum("chk,hk->ch", W1h, a_s1)
    V[:F_IN, 8:16] = np.einsum("chk,hk->ch", W1h, a_d1)
    w1pad = np.zeros((64, D), np.float32)
    w1pad[:F_IN] = W1f
    W2f = np.asarray(inputs["W2"], np.float32)
    w2s_h = np.stack([W2f @ np.asarray(inputs["a_src2"], np.float32)[0],
                      W2f @ np.asarray(inputs["a_dst2"], np.float32)[0]], axis=1)
    W3f = np.asarray(inputs["W3"], np.float32)
    w3e_h = np.concatenate(
        [W3f, (W3f @ np.asarray(inputs["a_src3"], np.float32)[0])[:, None],
         (W3f @ np.asarray(inputs["a_dst3"], np.float32)[0])[:, None]], axis=1)
    x_base = np.zeros((N, ROW1), ml_dtypes.bfloat16)
    x_base[:, :F_IN] = bf(x)
    x_t = np.zeros((64, N), ml_dtypes.bfloat16)
    x_t[:F_IN] = bf(x).T
    brow = np.concatenate([np.asarray(inputs["b1"], np.float32),
                           np.asarray(inputs["b2"], np.float32),
                           np.asarray(inputs["b3"], np.float32)])[None, :]
    shared = {
        "x_base": x_base, "x_t": x_t, "v1": bf(V), "w1p": bf(w1pad),
        "w2": bf(W2f), "w2s": bf(w2s_h), "w3e": bf(w3e_h), "brow": brow,
    }
    in_maps = []
    for c in range(NC):
        m = dict(shared)
        xtc = np.zeros((64, G * P), ml_dtypes.bfloat16)
        xtc[:, :NODES] = x_t[:, c * NODES:(c + 1) * NODES]
        m["xtc"] = xtc
        m["idx1"] = idx1_in[c]
        m["idx2"] = idx2_in[c]
        m["idx3"] = idx3_in[c]
        m["dlo"] = dlo_in[c]
        m["vb"] = vb_in[c].astype(ml_dtypes.bfloat16)
        in_maps.append(m)
    return in_maps


def kernel(x, edge_index, W1, a_src1, a_dst1, b1, W2, a_src2, a_dst2, b2,
           W3, a_src3, a_dst3, b3, stop_after=None):
    inputs = dict(x=x, edge_index=np.asarray(edge_index), W1=W1, a_src1=a_src1,
                  a_dst1=a_dst1, b1=b1, W2=W2, a_src2=a_src2, a_dst2=a_dst2, b2=b2,
                  W3=W3, a_src3=a_src3, a_dst3=a_dst3, b3=b3)
    K1, K2, i1, i2, i3, dlo_in, vb_in = _host_prep(inputs["edge_index"])

    key = (K1, K2, stop_after)
    if key not in _CACHE:
        nc = bacc.Bacc("TRN2", target_bir_lowering=False, debug=False, num_devices=NC)
        build(nc, K1, K2, stop_after=stop_after)
        nc.compile()
        _CACHE[key] = nc
    nc = _CACHE[key]

    in_maps = _prep_in_maps(inputs, K1, K2, i1, i2, i3, dlo_in, vb_in)
    res = None
    if int(os.environ.get("GAT_TRACE", "0")):
        try:
            res = run_bass_kernel_spmd(nc, in_maps, core_ids=list(range(NC)), trace=True)
        except Exception:
            res = None
    if res is None:
        res = run_bass_kernel_spmd(nc, in_maps, core_ids=list(range(NC)))
    global LAST_EXEC_NS
    LAST_EXEC_NS = res.exec_time_ns
    if stop_after is not None:
        return res
    out = np.concatenate([res.results[c]["out_shard"] for c in range(NC)], axis=0)
    return out.astype(np.float32)
